# revision 2
# baseline (speedup 1.0000x reference)
"""Trainium2 Bass kernel for nn_EnhancedEdgeScorer (gnn_message_passing), v2.

Sharding: data-parallel over nodes (2048/core) and edges (8192/core) on 8
NeuronCores.  Algebraic folds (as v1): k/v projected before the neighbor
gather; k-bias drops (softmax shift invariance); v-bias folds into the
out-projection bias; the 1/sqrt(dh) scale folds into wq/bq.  Nodes globally
sorted by context length, tiles dealt snake-wise to cores so the SPMD
per-tile m-extents are equal across cores.

v2 communication scheme (replaces v1's serial per-layer K|V AllGather):
  - The host precomputes x0, q0 AND kvall0 (= [x0@wk0 | x0@wv0], exact
    reference math in f32 -> bf16), so layer 1 starts gathering immediately.
  - Per layer, the layer's OUTPUT x is AllGathered in four quarter
    collectives (transposed layout, 2.1MB each) fired mid-layer as the
    producing tiles complete, so the collectives ride under the attention
    compute instead of serializing at the layer boundary.
  - Each core then projects K|V for ALL N nodes locally on the PE
    (x^T quarters arrive pre-transposed; 2 matmuls per 128-node group into
    fp32 PSUM, scalar-engine copy to bf16, batched DMA into a local kvall
    table).  The per-layer 16.8MB K|V collective disappears entirely.
  - The last layer AllGathers x in node-major quarters for the edge MLP
    (same as v1).
Gathers for the current layer are interleaved with the quarter collectives
on the Pool queue (a collective blocks the queue while in flight, so the
gather prefetch depth and the AG emission points are tuned to keep the DVE
fed).  Everything dense runs on the PE in bf16 with fp32 PSUM.
"""

import numpy as np
import ml_dtypes
from contextlib import ExitStack

import concourse.bass as bass
from concourse import bacc
import concourse.tile as tile
import concourse.mybir as mybir
from concourse.masks import make_identity
from concourse.bass_utils import run_bass_kernel_spmd

BF16 = mybir.dt.bfloat16
F8 = mybir.dt.float8e4
F32 = mybir.dt.float32
I16 = mybir.dt.int16

N, M, H, HEADS, L, E = 16384, 32, 256, 4, 3, 65536
DH = H // HEADS
NC = 8
NL = N // NC      # 2048 nodes per core
EL = E // NC      # 8192 edges per core
P = 128
NT = NL // P      # 16 node tiles per core
ET = EL // 512    # 16 edge chunks per core
NEG = -30.0       # additive pad-mask value
CH = (5, 5, 5, 1)             # tiles per AG chunk (last small: fires at layer end)
CHOFF = (0, 5, 10, 15)        # first tile of each chunk
QF = NL // 4                  # legacy (unused)

_bf = lambda a: np.ascontiguousarray(a.astype(ml_dtypes.bfloat16))
_f8 = lambda a: np.ascontiguousarray(a.astype(ml_dtypes.float8_e4m3))
_f32 = lambda a: np.ascontiguousarray(a.astype(np.float32))


def _wrap16(idx):
    idx = np.asarray(idx, dtype=np.int16)
    assert idx.size % 16 == 0
    return np.ascontiguousarray(np.tile(idx.reshape(-1, 16).T, (8, 1)))


# --------------------------------------------------------------------------
# Bass program (SPMD; per-core differences enter only through input data)
# --------------------------------------------------------------------------

def build_program(extents):
    extents = list(extents)
    stot = sum(extents)
    idxn = P * stot
    assert idxn % 16 == 0

    nc = bacc.Bacc(num_devices=NC)
    dp = lambda nm, shp, dt: nc.declare_dram_parameter(nm, list(shp), dt, isOutput=False)

    # ---- weights ----
    wqT = dp("wqT", [L, 2, P, H], BF16)                   # (wq*scale).T row-chunks
    bq = dp("bq", [L, H], F32)                            # bq*scale
    woT = dp("woT", [L, 2, P, H], BF16)
    bo = dp("bo", [L, H], F32)                            # out_b + out_w@bv
    wkv = dp("wkv", [2, 2, P, 2 * H], F8)               # [layer-1..2][feat chunk][feat, K|V out]
    w1T = dp("w1T", [4, P, H], BF16)
    w1eT = dp("w1eT", [2, H], BF16)
    b1 = dp("b1", [P, 2], F32)
    w2T = dp("w2T", [2, P, H // 2], BF16)
    b2 = dp("b2", [H // 2], F32)
    w3T = dp("w3T", [P, 1], BF16)
    b3 = dp("b3", [1], F32)

    # ---- per-core data ----
    idx_kv = dp("idx_kv", [P, idxn // 16], I16)
    idx_u = dp("idx_u", [P, EL // 16], I16)
    idx_v = dp("idx_v", [P, EL // 16], I16)
    kp = dp("kp", [P, stot], BF16)
    efT = dp("efT", [2, EL], BF16)
    q0 = dp("q0", [P, NT, H], BF16)                       # host layer-0 queries
    kvall0 = dp("kvall0", [N, 2 * H], BF16)               # host layer-0 K|V (pos_x order)

    out_d = nc.declare_dram_parameter("out", [EL], F32, isOutput=True)

    # ---- internal DRAM ----
    kvall1 = nc.dram_tensor("kvall1", [N, 2 * H], BF16)
    kvall2 = nc.dram_tensor("kvall2", [N, 2 * H], BF16)
    kvtabs = [kvall0, kvall1, kvall2]
    xlocT = [nc.dram_tensor(f"xlocT{c}", [2, P, CH[c] * P], F8) for c in range(4)]
    xloc = nc.dram_tensor("xloc", [NL, H], BF16)
    xallT = [nc.dram_tensor(f"xallT{c}", [NC, 2, P, CH[c] * P], F8,
                            addr_space="Shared") for c in range(4)]
    xall = nc.dram_tensor("xall", [N, H], BF16, addr_space="Shared")

    groups = [list(range(NC))]
    Alu = mybir.AluOpType
    Act = mybir.ActivationFunctionType

    with tile.TileContext(nc) as tc, ExitStack() as ctx:
        const = ctx.enter_context(tc.tile_pool(name="const", bufs=1))
        xpool = ctx.enter_context(tc.tile_pool(name="xpool", bufs=1))

        gather = nc.gpsimd.dma_gather
        reg_e2 = nc.gpsimd.to_reg(EL // 2)
        reg_pe = {}

        ident = const.tile([P, P], BF16)
        make_identity(nc, ident)

        def bcast_row(dram_ap, n, name):
            t = const.tile([P, n], F32, tag=name, name=name)
            src = bass.AP(tensor=dram_ap.tensor, offset=dram_ap.offset,
                          ap=[[0, P]] + dram_ap.ap)
            nc.sync.dma_start(out=t[:], in_=src)
            return t

        bq_b = [bcast_row(bq[ll, :], H, f"bq{ll}") for ll in range(L)]
        bo_b = [bcast_row(bo[ll, :], H, f"bo{ll}") for ll in range(L)]

        b1_sb = const.tile([P, 2], F32)
        nc.sync.dma_start(out=b1_sb[:], in_=b1[:])
        b2_sb = const.tile([H // 2, 1], F32)
        nc.sync.dma_start(out=b2_sb[:], in_=b2.rearrange("(p o) -> p o", o=1))
        b3_sb = const.tile([1, 1], F32)
        nc.sync.dma_start(out=b3_sb[:], in_=b3.rearrange("(p o) -> p o", o=1))

        ikv_sb = const.tile([P, idxn // 16], I16)
        nc.sync.dma_start(out=ikv_sb[:], in_=idx_kv[:])
        iu_sb = const.tile([P, EL // 16], I16)
        nc.sync.dma_start(out=iu_sb[:], in_=idx_u[:])
        iv_sb = const.tile([P, EL // 16], I16)
        nc.sync.dma_start(out=iv_sb[:], in_=idx_v[:])

        kp_sb = const.tile([P, stot], BF16)
        nc.sync.dma_start(out=kp_sb[:], in_=kp[:])

        w1_sb = const.tile([P, 4, H], BF16)
        nc.sync.dma_start(out=w1_sb[:], in_=w1T.rearrange("c p o -> p c o"))
        w1e_sb = const.tile([2, H], BF16)
        nc.sync.dma_start(out=w1e_sb[:], in_=w1eT[:])
        w2_sb = const.tile([P, 2, H // 2], BF16)
        nc.sync.dma_start(out=w2_sb[:], in_=w2T.rearrange("c p o -> p c o"))
        w3_sb = const.tile([P, 1], BF16)
        nc.sync.dma_start(out=w3_sb[:], in_=w3T[:])

        # kv projection weights for layers 1, 2 (both loaded up front)
        wkv_sb = const.tile([P, 2, 2, 2 * H], F8)
        nc.sync.dma_start(out=wkv_sb[:], in_=wkv.rearrange("l c p o -> p l c o"))

        x_sb = xpool.tile([P, NT, H], BF16)

        work = ctx.enter_context(tc.tile_pool(name="work", bufs=1))
        gath = ctx.enter_context(tc.tile_pool(name="gath", bufs=2))
        att = ctx.enter_context(tc.tile_pool(name="att", bufs=2))
        big = ctx.enter_context(tc.tile_pool(name="big", bufs=1))
        proj = ctx.enter_context(tc.tile_pool(name="proj", bufs=2))
        psum = ctx.enter_context(tc.tile_pool(name="psum", bufs=2, space="PSUM"))
        psum1 = ctx.enter_context(tc.tile_pool(name="psum1", bufs=2, space="PSUM"))

        moffs = []
        mo = 0
        for e in extents:
            moffs.append(mo)
            mo += e
        # greedy gather groups (adjacent tiles share one dma_gather while the
        # summed extent fits the 32-slot buffer)
        ggroups = []
        cur, ce = [], 0
        for t, e in enumerate(extents):
            if cur and ce + e > M:
                ggroups.append(cur)
                cur, ce = [], 0
            cur.append(t)
            ce += e
        ggroups.append(cur)
        gleader = {}
        for grp in ggroups:
            ge = sum(extents[t] for t in grp)
            off = 0
            for t in grp:
                gleader[t] = (grp[0], ge, off)
                off += extents[t]
        for grp in ggroups:
            ge = sum(extents[t] for t in grp)
            if ge not in reg_pe:
                reg_pe[ge] = nc.gpsimd.to_reg(P * ge)

        # ---------------- attention pipeline pieces ----------------
        xT = work.tile([P, 2, NT, P], BF16, tag="xT", name="xT")

        def load_w(tag, dram, ll):
            tW = work.tile([P, 2, H], BF16, tag=tag, name=f"{tag}{ll}", bufs=2)
            nc.sync.dma_start(out=tW[:], in_=dram[ll].rearrange("c p o -> p c o"))
            return tW

        gbufs = {}
        gcount = [0]
        gwait = {}

        def stage1(ll, t, q_sb):
            e = extents[t]
            moff = moffs[t]
            leader, ge, goff = gleader[t]
            if t == leader:
                kv_f = gath.tile([P, ge * 2 * H], BF16, tag="kvg", name="kv_f")
                isl = ikv_sb[:, moff * P // 16:(moff + ge) * P // 16]
                gsem = nc.alloc_semaphore(f"gsem{ll}_{leader}")
                gather(kv_f.rearrange("p (m o) -> p m o", o=2 * H), kvtabs[ll][:],
                       isl, P * ge, reg_pe[ge], 2 * H, single_packet=False,
                       prepare_only=True, sem=gsem)
                nc.gpsimd.trigger_dma(count=1)
                gwait[leader] = gsem
                gbufs[leader] = kv_f
            kvg = gbufs[leader][:, goff * 2 * H:(goff + e) * 2 * H].rearrange(
                "p (m o) -> p m o", o=2 * H)
            if t == leader:
                nc.vector.wait_ge(gwait[leader], 16)
            pp_f = big.tile([P, e * H], BF16, tag="pp", name="pp")
            pp = pp_f.rearrange("p (m h d) -> p m h d", h=HEADS, d=DH)
            qb = q_sb[:, t, None, :].to_broadcast([P, e, H])
            nc.vector.tensor_tensor(pp.rearrange("p m h d -> p m (h d)"),
                                    kvg[:, :, 0:H], qb, op=Alu.mult)
            d = DH
            while d > 2:
                d //= 2
                nc.vector.tensor_tensor(pp[:, :, :, 0:d], pp[:, :, :, 0:d],
                                        pp[:, :, :, d:2 * d], op=Alu.add)
            s_m = att.tile([P, e, HEADS], BF16, tag="sm", name="s_m")
            nc.vector.tensor_tensor(s_m[:], pp[:, :, :, 0], pp[:, :, :, 1], op=Alu.add)
            kpb = kp_sb[:, moff:moff + e, None].to_broadcast([P, e, HEADS])
            nc.vector.tensor_tensor(s_m[:], s_m[:], kpb, op=Alu.add)
            ex_f = big.tile([P, e * H], BF16, tag="esx", name="ex_f")
            es_x = ex_f.rearrange("p (m h d) -> p m h d", h=HEADS, d=DH)
            nc.scalar.activation(es_x[:], s_m[:, :, :, None].to_broadcast([P, e, HEADS, DH]),
                                 Act.Exp)
            return (t, e, kvg, ex_f)

        def stage2a(ll, st, wo_sb):
            t, e, kvg, ex_f = st
            es_x = ex_f.rearrange("p (m h d) -> p m h d", h=HEADS, d=DH)
            sums = att.tile([P, HEADS], F32, tag="sums", name="sums")
            nc.vector.tensor_reduce(sums[:], es_x[:, :, :, 0].rearrange("p m h -> p h m"),
                                    axis=mybir.AxisListType.X, op=Alu.add)
            rs = att.tile([P, HEADS], F32, tag="rs", name="rs")
            nc.vector.reciprocal(rs[:], sums[:])
            av_f = big.tile([P, e * H], BF16, tag="av", name="av")
            av = av_f.rearrange("p (m o) -> p m o", o=H)
            nc.vector.tensor_tensor(av[:], kvg[:, :, H:2 * H],
                                    ex_f.rearrange("p (m o) -> p m o", o=H), op=Alu.mult)
            m = e
            p2 = 1
            while p2 * 2 <= m:
                p2 *= 2
            if m > p2:
                nc.vector.tensor_tensor(av[:, 0:m - p2, :], av[:, 0:m - p2, :],
                                        av[:, p2:m, :], op=Alu.add)
                m = p2
            while m > 1:
                m //= 2
                nc.vector.tensor_tensor(av[:, 0:m, :], av[:, 0:m, :],
                                        av[:, m:2 * m, :], op=Alu.add)
            o_sb = att.tile([P, HEADS, DH], BF16, tag="o", name="o_sb")
            nc.vector.tensor_tensor(o_sb[:], av[:, 0, :].rearrange("p (h d) -> p h d", h=HEADS),
                                    rs[:, :, None].to_broadcast([P, HEADS, DH]), op=Alu.mult)
            oT = att.tile([P, 2, P], BF16, tag="oT", name="oT")
            for c in range(2):
                pt = psum1.tile([P, P], BF16, tag="ptr", name="pt")
                nc.tensor.transpose(pt[:], o_sb.rearrange("p h d -> p (h d)")[:, c * P:(c + 1) * P],
                                    ident[:])
                nc.scalar.activation(oT[:, c, :], pt[:], Act.Copy)
            pxn = psum.tile([P, H], F32, tag="pxn", name="pxn", bufs=2)
            nc.tensor.matmul(pxn[:], oT[:, 0, :], wo_sb[:, 0, :], start=True, stop=False)
            nc.tensor.matmul(pxn[:], oT[:, 1, :], wo_sb[:, 1, :], start=False, stop=True)
            return pxn

        def stage2b(ll, t, pxn):
            nc.vector.tensor_tensor(x_sb[:, t, :], pxn[:], bo_b[ll][:], op=Alu.add)
            nc.vector.tensor_scalar_max(x_sb[:, t, :], x_sb[:, t, :], 0.0)

        xloc_pv = xloc.rearrange("(t p) o -> p t o", p=P)
        xall_cv = xall

        # ---- tails: transposes + next-layer q + AG staging writes ----
        def tail(ll, t, wq_nxt, q_nxt):
            if ll < L - 1:
                t8 = work.tile([P, 2, P], F8, tag="xT8", name=f"t8_{ll}_{t}", bufs=2)
                for c in range(2):
                    pt = psum1.tile([P, P], BF16, tag="ptr", name="pt")
                    nc.tensor.transpose(pt[:], x_sb[:, t, c * P:(c + 1) * P], ident[:])
                    nc.scalar.activation(xT[:, c, t, :], pt[:], Act.Copy)
                    nc.scalar.activation(t8[:, c, :], pt[:], Act.Copy)
                pq = psum.tile([P, H], F32, tag="pmm", name="pq")
                nc.tensor.matmul(pq[:], xT[:, 0, t, :], wq_nxt[:, 0, :], start=True, stop=False)
                nc.tensor.matmul(pq[:], xT[:, 1, t, :], wq_nxt[:, 1, :], start=False, stop=True)
                nc.vector.tensor_tensor(q_nxt[:, t, :], pq[:], bq_b[ll + 1][:], op=Alu.add)
                # stage fp8 x^T chunk slice for the AG
                q = max(c for c in range(4) if CHOFF[c] <= t)
                j = t - CHOFF[q]
                nc.sync.dma_start(
                    out=xlocT[q][:, :, j * P:(j + 1) * P].rearrange("c p n -> p c n"),
                    in_=t8[:])
            else:
                # node-major staging for the edge-phase x table
                for qq in range(4):
                    if t + 1 == CHOFF[qq] + CH[qq]:
                        t0c = CHOFF[qq]
                        nc.sync.dma_start(out=xloc_pv[:, t0c:t0c + CH[qq], :],
                                          in_=x_sb[:, t0c:t0c + CH[qq], :])

        # ---- per-quarter collectives + all-N kv projection ----
        choff_g = [0]
        for c in range(4):
            choff_g.append(choff_g[-1] + CH[c] * P * NC)

        def emit_ag(ll, q):
            if ll < L - 1:
                nc.gpsimd.collective_compute(
                    "AllGather", Alu.bypass, replica_groups=groups,
                    ins=[xlocT[q][:]],
                    outs=[xallT[q].rearrange("r c p n -> (r c p) n").rearrange(
                        "(a b) n -> a b n", a=NC)])
            else:
                t0c, nl_c = CHOFF[q] * P, CH[q] * P
                nc.gpsimd.collective_compute(
                    "AllGather", Alu.bypass, replica_groups=groups,
                    ins=[xloc[t0c:t0c + nl_c, :]],
                    outs=[xall[choff_g[q]:choff_g[q + 1], :].rearrange(
                        "(a b) o -> a b o", a=nl_c)])

        def load_xs(q, r):
            """Stage gathered x^T of (chunk q, rank r) into SBUF (SP queue)."""
            xs = proj.tile([P, 2, 5 * P], F8, tag="xs", name=f"xs{q}_{r}", bufs=4)
            nn = CH[q] * P
            nc.sync.dma_start(out=xs[:, :, 0:nn],
                              in_=xallT[q][r].rearrange("c p n -> p c n"))
            return xs

        def do_proj(ll, q, r, xs, kvnext):
            """Project CH[q]*128 gathered nodes (chunk q, rank r) -> kvnext rows."""
            stg = proj.tile([P, 5, 2 * H], BF16, tag="stg", name="stg")
            for g in range(CH[q]):
                pk = psum.tile([P, 2 * H], F32, tag="pbig", name="pk", bufs=2)
                gs = slice(g * P, (g + 1) * P)
                nc.tensor.matmul(pk[:], xs[:, 0, gs], wkv_sb[:, ll, 0, :],
                                 start=True, stop=False)
                nc.tensor.matmul(pk[:], xs[:, 1, gs], wkv_sb[:, ll, 1, :],
                                 start=False, stop=True)
                nc.scalar.activation(stg[:, g, :], pk[:], Act.Copy)
            nl_c = CH[q] * P
            r0 = choff_g[q] + r * nl_c
            nc.sync.dma_start(
                out=kvnext[r0:r0 + nl_c, :].rearrange("(g p) o -> p g o", p=P),
                in_=stg[:, 0:CH[q], :])

        # AG emission step per chunk (chunk q's last staging write happens in
        # tail(CHOFF[q]+CH[q]-1) at loop step +2; emit one step later).
        ag_at = {8: 0, 13: 1, 17: 2}

        q_cur = work.tile([P, NT, H], BF16, tag="q", name="q0in", bufs=2)
        nc.sync.dma_start(out=q_cur[:], in_=q0[:])

        for ll in range(L):
            wo_sb = load_w("wo", woT, ll)
            if ll < L - 1:
                wq_sb = load_w("wq", wqT, ll + 1)
                q_nxt = work.tile([P, NT, H], BF16, tag="q", name=f"q{ll + 1}", bufs=2)
            else:
                wq_sb, q_nxt = None, None
            kvnext = [kvall1, kvall2, None][ll]

            ag_emitted = -1
            pend_load = []   # (q, r) whose xs load not yet emitted
            xs_tiles = {}    # (q, r) -> staged xs tile
            pend_proj = []   # (q, r) staged, matmuls not yet emitted

            def drain_loads(upto_q):
                while pend_load and pend_load[0][0] <= upto_q:
                    qq, rr = pend_load.pop(0)
                    xs_tiles[(qq, rr)] = load_xs(qq, rr)
                    pend_proj.append((qq, rr))

            def drain_proj(k, upto_q):
                while pend_proj and k > 0 and pend_proj[0][0] <= upto_q:
                    qq, rr = pend_proj.pop(0)
                    do_proj(ll, qq, rr, xs_tiles.pop((qq, rr)), kvnext)
                    k -= 1

            sts = {}
            pxns = {}
            for t in range(NT + 2):
                if t < NT:
                    sts[t] = stage1(ll, t, q_cur)
                if t >= 1 and t - 1 < NT:
                    pxns[t - 1] = stage2a(ll, sts.pop(t - 1), wo_sb)
                if t >= 2:
                    stage2b(ll, t - 2, pxns.pop(t - 2))
                    tail(ll, t - 2, wq_sb, q_nxt)
                if t in ag_at:
                    q = ag_at[t]
                    emit_ag(ll, q)
                    ag_emitted = q
                    if ll < L - 1:
                        pend_load.extend((q, r) for r in range(NC))
                if ll < L - 1:
                    # xs loads trail the AG emission (SP-queue waits are
                    # harmless); matmuls trail by one chunk so the PE never
                    # stalls on an in-flight collective
                    drain_loads(ag_emitted)
                    drain_proj(3, ag_emitted - 1)
            # final quarter: AG after the last tail, then drain everything
            emit_ag(ll, 3)
            if ll < L - 1:
                pend_load.extend((3, r) for r in range(NC))
                drain_loads(2)
                drain_proj(NC + len(pend_proj), 2)
                drain_loads(3)
                drain_proj(NC + len(pend_proj), 3)
            if ll < L - 1:
                q_cur = q_nxt

        # ---------------- edge MLP ----------------
        zero1 = const.tile([P, 1], F32)
        nc.vector.memset(zero1[:], 0.0)
        EH = EL // 2
        etags = [("pp", "esx"), ("av", "kvg")]
        ugs, vgs = [], []
        for half in range(2):
            hsl = slice(half * (EH // 16), (half + 1) * (EH // 16))
            ug = big.tile([P, 2, EH], BF16, tag=etags[half][0], name=f"ug{half}")
            gather(ug[:], xall[:], iu_sb[:, hsl], EH, reg_e2, H,
                   transpose=True, single_packet=False)
            vg2 = (big.tile([P, 2, EH], BF16, tag=etags[half][1], name=f"vg{half}")
                   if half == 0 else
                   gath.tile([P, 2, EH], BF16, tag=etags[half][1], name=f"vg{half}"))
            gather(vg2[:], xall[:], iv_sb[:, hsl], EH, reg_e2, H,
                   transpose=True, single_packet=False)
            ugs.append(ug)
            vgs.append(vg2)
        for half in range(2):
            ug, vg2 = ugs[half], vgs[half]
            for e in range(EH // 512):
                eg = half * (EH // 512) + e
                esl = slice(e * 512, (e + 1) * 512)
                ef_sb = att.tile([2, 512], BF16, tag="ef", name="ef_sb")
                nc.sync.dma_start(out=ef_sb[:], in_=efT[:, eg * 512:(eg + 1) * 512])
                h1T = att.tile([P, 2, 512], BF16, tag="h1T", name="h1T")
                for oc in range(2):
                    ph = psum.tile([P, 512], F32, tag="pbig", name="ph", bufs=2)
                    ocs = slice(oc * P, (oc + 1) * P)
                    nc.tensor.matmul(ph[:], w1_sb[:, 0, ocs], ug[:, 0, esl], start=True, stop=False)
                    nc.tensor.matmul(ph[:], w1_sb[:, 1, ocs], ug[:, 1, esl], start=False, stop=False)
                    nc.tensor.matmul(ph[:], w1_sb[:, 2, ocs], vg2[:, 0, esl], start=False, stop=False)
                    nc.tensor.matmul(ph[:], w1_sb[:, 3, ocs], vg2[:, 1, esl], start=False, stop=False)
                    nc.tensor.matmul(ph[:], w1e_sb[:, ocs], ef_sb[:], start=False, stop=True)
                    nc.scalar.activation(h1T[:, oc, :], ph[:], Act.Relu, bias=b1_sb[:, oc:oc + 1])
                ph2 = psum.tile([P, 512], F32, tag="pbig", name="ph2", bufs=2)
                nc.tensor.matmul(ph2[0:H // 2, :], w2_sb[:, 0, :], h1T[:, 0, :], start=True, stop=False)
                nc.tensor.matmul(ph2[0:H // 2, :], w2_sb[:, 1, :], h1T[:, 1, :], start=False, stop=True)
                h2T = att.tile([H // 2, 512], BF16, tag="h2T", name="h2T")
                nc.vector.scalar_tensor_tensor(h2T[:], ph2[0:H // 2, :], b2_sb[:],
                                               zero1[0:H // 2, :].to_broadcast([H // 2, 512]),
                                               op0=Alu.add, op1=Alu.max)
                pl = psum.tile([1, 512], F32, tag="pxn", name="pl", bufs=2)
                nc.tensor.matmul(pl[:], w3_sb[:, :], h2T[:], start=True, stop=True)
                lo = att.tile([1, 512], F32, tag="lo", name="lo")
                nc.vector.tensor_scalar_add(lo[:], pl[:], b3_sb[:])
                nc.sync.dma_start(out=out_d.rearrange("(a b) -> a b", a=ET)[eg, None, :], in_=lo[:])

    nc.finalize()
    return nc


# --------------------------------------------------------------------------
# Host-side prep + runner
# --------------------------------------------------------------------------

_CACHE = {}


def _plan(key_padding_mask):
    kpm = np.asarray(key_padding_mask, dtype=bool)
    lens = M - kpm.sum(1)
    order = np.argsort(lens, kind="stable")
    tiles = order.reshape(N // P, P)
    extents = []
    for r in range(NT):
        grp = tiles[r * NC:(r + 1) * NC]
        extents.append(int(max(1, lens[grp].max())))
    perm = np.concatenate([
        np.concatenate([tiles[r * NC + c] for r in range(NT)]) for c in range(NC)
    ])
    return perm, tuple(extents)


def _prep_maps(inputs, perm, extents):
    f = {k: np.asarray(v) for k, v in inputs.items()}
    scale = 1.0 / np.sqrt(np.float32(DH))
    slot = np.empty(N, np.int64)
    slot[perm] = np.arange(N)
    cc, ii = slot // NL, slot % NL
    rr, pp = ii // P, ii % P
    choff_g = np.zeros(5, np.int64)
    for c in range(4):
        choff_g[c + 1] = choff_g[c] + CH[c] * P * NC
    qq = np.zeros_like(rr)
    for c in range(1, 4):
        qq = np.where(rr >= CHOFF[c], c, qq)
    pos_x = (choff_g[qq] + cc * (np.array(CH)[qq] * P)
             + (rr - np.array(CHOFF)[qq]) * P + pp)

    wqT = np.empty((L, 2, P, H), np.float32)
    woT = np.empty((L, 2, P, H), np.float32)
    bqv = np.empty((L, H), np.float32)
    bov = np.empty((L, H), np.float32)
    wk_l = []
    wv_l = []
    for ll in range(L):
        w = f["in_proj_w"][ll].astype(np.float32)
        b = f["in_proj_b"][ll].astype(np.float32)
        wq, wk, wv = w[0:H], w[H:2 * H], w[2 * H:3 * H]
        wk_l.append(wk)
        wv_l.append(wv)
        bqv[ll] = b[0:H] * scale
        bv = b[2 * H:3 * H]
        for c in range(2):
            wqT[ll, c] = (wq * scale).T[c * P:(c + 1) * P]
            woT[ll, c] = f["out_w"][ll].astype(np.float32).T[c * P:(c + 1) * P]
        bov[ll] = f["out_b"][ll].astype(np.float32) + f["out_w"][ll].astype(np.float32) @ bv

    # moving-layout K|V projection weights for layers 1, 2:
    # wkv[l-1, c] = [wk_l.T | wv_l.T][c*128:(c+1)*128]  -> [128, 512]
    wkv = np.empty((2, 2, P, 2 * H), np.float32)
    for li, ll in enumerate((1, 2)):
        kvT = np.concatenate([wk_l[ll].T, wv_l[ll].T], axis=1)  # [256, 512]
        for c in range(2):
            wkv[li, c] = kvT[c * P:(c + 1) * P]

    w1 = f["mlp_w1"].astype(np.float32)
    w1T_full = w1.T
    w1T = np.stack([w1T_full[c * P:(c + 1) * P] for c in range(4)])
    w1eT = w1T_full[512:514]
    b1 = f["mlp_b1"].astype(np.float32).reshape(2, P).T
    w2T = np.stack([f["mlp_w2"].astype(np.float32).T[c * P:(c + 1) * P] for c in range(2)])
    w3T = f["mlp_w3"].astype(np.float32).T

    # host layer-0 encode + projections (exact reference math in f32)
    te = f["type_embed"].astype(np.float32)[f["type_idx"]]
    ce = np.concatenate([f["cat_embed0"].astype(np.float32)[f["cat_idx"][:, 0]],
                         f["cat_embed1"].astype(np.float32)[f["cat_idx"][:, 1]]], 1)
    de = np.maximum(f["log_degree"].astype(np.float32)
                    @ f["degree_w"].astype(np.float32).T
                    + f["degree_b"].astype(np.float32), 0.0)
    x0 = (np.concatenate([te, ce, de], 1) @ f["proj_w"].astype(np.float32).T
          + f["proj_b"].astype(np.float32))
    x0 = np.asarray(x0.astype(ml_dtypes.bfloat16)).astype(np.float32)  # device bf16 x
    # layer-0 K|V for all N in pos_x order
    kv0 = np.concatenate([x0 @ wk_l[0].T, x0 @ wv_l[0].T], axis=1)
    kvall0 = np.empty((N, 2 * H), np.float32)
    kvall0[pos_x] = kv0
    # layer-0 queries (scaled) per core
    q0_full = x0 @ (f["in_proj_w"][0][0:H].astype(np.float32) * scale).T + bqv[0]

    shared = {
        "wqT": _bf(wqT), "bq": _f32(bqv),
        "woT": _bf(woT), "bo": _f32(bov),
        "wkv": _f8(wkv),
        "w1T": _bf(w1T), "w1eT": _bf(w1eT), "b1": _f32(b1),
        "w2T": _bf(w2T), "b2": _f32(f["mlp_b2"]),
        "w3T": _bf(w3T), "b3": _f32(f["mlp_b3"]),
        "kvall0": _bf(kvall0),
    }

    ctx = f["context_indices"].astype(np.int64)
    ctx_pos = pos_x[ctx]
    kpm = f["key_padding_mask"].astype(bool)
    maps = []
    for c in range(NC):
        ns = slice(c * NL, (c + 1) * NL)
        es = slice(c * EL, (c + 1) * EL)
        nodes = perm[ns]
        idx_kv = []
        kp_c = np.empty((P, sum(extents)), np.float32)
        moff = 0
        for r in range(NT):
            tl = nodes[r * P:(r + 1) * P]
            e = extents[r]
            idx_kv.append(ctx_pos[tl, :e].T.flatten())
            kp_c[:, moff:moff + e] = np.where(kpm[tl, :e], NEG, 0.0)
            moff += e
        m = dict(shared)
        m["idx_kv"] = _wrap16(np.concatenate(idx_kv))
        m["idx_u"] = _wrap16(pos_x[f["u_idx"][es]])
        m["idx_v"] = _wrap16(pos_x[f["v_idx"][es]])
        m["kp"] = _bf(kp_c)
        m["efT"] = _bf(f["edge_feats"][es].T)
        # q0 for own nodes: [P, NT, H] with q0[p, t] = q of node perm[c*NL + t*P + p]
        qrows = q0_full[nodes].reshape(NT, P, H).transpose(1, 0, 2)
        m["q0"] = _bf(qrows)
        maps.append(m)
    return maps


def kernel(**inputs):
    perm, extents = _plan(inputs["key_padding_mask"])
    if extents not in _CACHE:
        _CACHE[extents] = build_program(extents)
    nc = _CACHE[extents]
    maps = _prep_maps(inputs, perm, extents)
    res = run_bass_kernel_spmd(nc, maps, core_ids=list(range(NC)))
    return np.concatenate([res.results[c]["out"] for c in range(NC)]).astype(np.float32)


if __name__ == "__main__":
    import reference
    inputs = {k: np.asarray(v) for k, v in reference.setup_inputs().items()}
    perm, extents = _plan(inputs["key_padding_mask"])
    print("extents:", extents, "sum:", sum(extents))
    nc = build_program(extents)
    print("program built OK")
